# revision 8
# baseline (speedup 1.0000x reference)
"""Multi-head attention (B=4, N=2048, DIM=1024, H=16) on 8 Trainium2 cores.

Sharding: core c handles batch b = c//2 and head-group g = c%2 (8 heads,
channel slice g*512:(g+1)*512). No collectives: each core produces a partial
out-projection Y_part = attn_out_g @ Wo[:, g-slice].T; the host sums the two
partials per batch and adds the constant vector (bv @ Wo.T + bo), exploiting
  softmax(S) @ (V0 + 1 bv^T) Wo^T = softmax(S) V0 Wo^T + (bv Wo^T).
The K-projection bias is dropped entirely (softmax row-shift invariance).

On-core dataflow (per core), all matmuls in float32r (fp32 w/ 11-bit
mantissa, 4x faster than fp32 on the PE; inputs pre-rounded on host):
  K^T = Wk_g^T X_k^T          [512, 2048]  (d-major "head transposed")
  Q^T = Wq_g^T X_q^T + bq     [512, 2048]  (scale 1/8 folded into Wq, bq)
  V   = [X_v Wv_g^T | 1]      [2048, 8, 65] (token-major, ones column)
  per (q-block 512, head-pair): S^T[k,q] via row-paired K=64 matmuls,
  exp on ACT (PSUM->SBUF [128,1024] insts; no max subtraction needed:
  scores ~ N(0,1)), AV as M=65 matmuls (ones column gives the softmax
  denominator at PSUM partition 64; fp32r matmul dst must start at
  partition 0), normalize via DVE reciprocal + gpsimd partition
  broadcast + DVE mul, then Y_part = O^T-as-weights @ Wo.
"""

import numpy as np

import concourse.bacc as bacc
import concourse.bass as bass
import concourse.mybir as mybir
import concourse.tile as tile
from concourse.bass_utils import run_bass_kernel_spmd

P = 128
B, N, DIM, H, DH = 4, 2048, 1024, 16, 64
SCALE = DH ** -0.5
CD = DIM // 2          # per-core channel slice (8 heads)
HG = CD // DH          # heads per core = 8
KT8 = DIM // P         # 8 contraction tiles for projections
CT4 = CD // P          # 4 c'-tiles (= head pairs)
QBN = N // 512         # 4 q-blocks
KTN = N // P           # 16 key tiles
F32R = mybir.dt.float32r
F32 = mybir.dt.float32
EXP = mybir.ActivationFunctionType.Exp


def _round_f32r(x: np.ndarray) -> np.ndarray:
    """Round fp32 to the FP32R format (11-bit mantissa, RNE) on the host."""
    b = np.ascontiguousarray(x, dtype=np.float32).view(np.uint32)
    lsb = (b >> np.uint32(12)) & np.uint32(1)
    r = (b + np.uint32(0x7FF) + lsb) & np.uint32(0xFFFFF000)
    return r.view(np.float32)


def _build(reps: int = 1, loop: bool = False):
    nc = bacc.Bacc("TRN2", target_bir_lowering=False, debug=False, num_devices=8)
    if loop:
        nreps = nc.dram_tensor("nreps", [1, 1], mybir.dt.int32, kind="ExternalInput")
    xq = nc.dram_tensor("xq", [DIM, N], F32R, kind="ExternalInput")
    xk = nc.dram_tensor("xk", [DIM, N], F32R, kind="ExternalInput")
    xv = nc.dram_tensor("xv", [DIM, N], F32R, kind="ExternalInput")
    wqt = nc.dram_tensor("wqt", [DIM, CD], F32R, kind="ExternalInput")
    wkt = nc.dram_tensor("wkt", [DIM, CD], F32R, kind="ExternalInput")
    wvt = nc.dram_tensor("wvt", [DIM, CD], F32R, kind="ExternalInput")
    wot = nc.dram_tensor("wot", [CD, DIM], F32R, kind="ExternalInput")
    bqs = nc.dram_tensor("bqs", [CD], F32, kind="ExternalInput")
    ones = nc.dram_tensor("ones", [1, KTN * HG], F32R, kind="ExternalInput")
    y = nc.dram_tensor("y", [N, DIM], F32, kind="ExternalOutput")

    with tile.TileContext(nc) as tc:
        with (
            tc.tile_pool(name="const", bufs=1) as const_pool,
            tc.tile_pool(name="kt", bufs=1) as kt_pool,
            tc.tile_pool(name="vt", bufs=1) as v_pool,
            tc.tile_pool(name="qt", bufs=1) as qt_pool,
        ):
            bq_sb = const_pool.tile([P, CT4], F32)
            nc.sync.dma_start(bq_sb[:], bqs.ap().rearrange("(t p) -> p t", p=P))
            kt_sb = kt_pool.tile([P, CT4, N], F32R)
            v_sb = v_pool.tile([P, KTN, HG, DH + 1], F32R)
            # ones column of V_aug (softmax denominator weights)
            nc.sync.dma_start(v_sb[:, :, :, DH:DH + 1],
                              ones.ap().to_broadcast((P, KTN * HG)))
            qt_sb = qt_pool.tile([P, CT4, N], F32R)

            if loop:
                nr_sb = const_pool.tile([1, 1], mybir.dt.int32)
                nc.sync.dma_start(nr_sb[:], nreps.ap())
                rv = nc.values_load(nr_sb[:], min_val=1, max_val=100000,
                                    skip_runtime_bounds_check=True)
                with tc.For_i(0, rv, 1):
                    _emit_once(nc, tc, xq, xk, xv, wqt, wkt, wvt, wot, y,
                               bq_sb, kt_sb, v_sb, qt_sb)
            else:
                for _ in range(reps):
                    _emit_once(nc, tc, xq, xk, xv, wqt, wkt, wvt, wot, y,
                               bq_sb, kt_sb, v_sb, qt_sb)
    nc.compile()
    return nc


def _emit_once(nc, tc, xq, xk, xv, wqt, wkt, wvt, wot, y,
               bq_sb, kt_sb, v_sb, qt_sb):
    # ---------------- projections: K, V, Q (n-half pipelined) -------------
    with (
        tc.tile_pool(name="xin", bufs=2) as x_pool,
        tc.tile_pool(name="win", bufs=1) as w_pool,
        tc.tile_pool(name="pps", bufs=4, space="PSUM") as proj_ps,
    ):
        # --- K projection: kt_sb[p, m, n] = (Wk^T Xk^T)[m*128+p, n]
        wk_sb = w_pool.tile([P, KT8, CD], F32R, tag="w")
        nc.sync.dma_start(wk_sb[:], wkt.ap().rearrange("(t p) m -> p t m", p=P))
        for nh in range(2):
            xh = x_pool.tile([P, KT8, N // 2], F32R, tag="x")
            nc.sync.dma_start(
                xh[:], xk.ap().rearrange("(t p) n -> p t n", p=P)[:, :, nh * (N // 2):(nh + 1) * (N // 2)])
            for m in range(CT4):
                for nb in range(2):
                    ps = proj_ps.tile([P, 512], F32)
                    for kk in range(KT8):
                        nc.tensor.matmul(ps[:], wk_sb[:, kk, m * P:(m + 1) * P],
                                         xh[:, kk, nb * 512:(nb + 1) * 512],
                                         start=(kk == 0), stop=(kk == KT8 - 1))
                    nabs = nh * (N // 2) + nb * 512
                    nc.any.tensor_copy(kt_sb[:, m, nabs:nabs + 512], ps[:])
        # --- V projection: v_sb[p, tt, h, d] = (Xv Wv^T)[tt*128+p, h*64+d]
        wv_sb = w_pool.tile([P, KT8, CD], F32R, tag="w")
        nc.sync.dma_start(wv_sb[:], wvt.ap().rearrange("(t p) m -> p t m", p=P))
        for nh in range(2):
            xh = x_pool.tile([P, KT8, N // 2], F32R, tag="x")
            nc.sync.dma_start(
                xh[:], xv.ap().rearrange("(t p) n -> p t n", p=P)[:, :, nh * (N // 2):(nh + 1) * (N // 2)])
            for tl in range(8):
                tt = nh * 8 + tl
                ps = proj_ps.tile([P, 512], F32)
                for kk in range(KT8):
                    nc.tensor.matmul(ps[:], xh[:, kk, tl * P:(tl + 1) * P],
                                     wv_sb[:, kk, :],
                                     start=(kk == 0), stop=(kk == KT8 - 1))
                nc.any.tensor_copy(v_sb[:, tt, :, 0:DH], ps[:])
        # --- Q projection (scaled weights; bias added at eviction)
        wq_sb = w_pool.tile([P, KT8, CD], F32R, tag="w")
        nc.sync.dma_start(wq_sb[:], wqt.ap().rearrange("(t p) m -> p t m", p=P))
        for nh in range(2):
            xh = x_pool.tile([P, KT8, N // 2], F32R, tag="x")
            nc.sync.dma_start(
                xh[:], xq.ap().rearrange("(t p) n -> p t n", p=P)[:, :, nh * (N // 2):(nh + 1) * (N // 2)])
            for nb in range(2):
                for m in range(CT4):
                    ps = proj_ps.tile([P, 512], F32)
                    for kk in range(KT8):
                        nc.tensor.matmul(ps[:], wq_sb[:, kk, m * P:(m + 1) * P],
                                         xh[:, kk, nb * 512:(nb + 1) * 512],
                                         start=(kk == 0), stop=(kk == KT8 - 1))
                    nabs = nh * (N // 2) + nb * 512
                    nc.vector.tensor_scalar_add(qt_sb[:, m, nabs:nabs + 512], ps[:],
                                                bq_sb[:, m:m + 1])

    # ---------------- attention + out-projection --------------------------
    with (
        tc.tile_pool(name="wo", bufs=1) as wo_pool,
        tc.tile_pool(name="pt", bufs=6) as p_pool,
        tc.tile_pool(name="ot", bufs=2) as ot_pool,
        tc.tile_pool(name="ysb", bufs=3) as y_pool,
        tc.tile_pool(name="rc", bufs=3) as r_pool,
        tc.tile_pool(name="rcb", bufs=3) as rb_pool,
        tc.tile_pool(name="sps", bufs=2, space="PSUM") as s_ps,
        tc.tile_pool(name="avps", bufs=2, space="PSUM") as av_ps,
        tc.tile_pool(name="yps", bufs=2, space="PSUM") as y_ps,
    ):
        wo_sb = wo_pool.tile([P, CT4, DIM], F32R)
        nc.sync.dma_start(wo_sb[:], wot.ap().rearrange("(t p) m -> p t m", p=P))
        for qb in range(QBN):
            q0 = qb * 512
            ot_t = ot_pool.tile([P, CT4, 512], F32R)
            for pr in range(CT4):
                avs = [av_ps.tile([P, 512], F32, tag="av", name=f"av{_h}")
                       for _h in range(2)]
                for kt in range(KTN):
                    ss = s_ps.tile([P, 2, 512], F32)
                    for hh in range(2):
                        p0 = hh * 64
                        nc.tensor.matmul(
                            ss[:, hh, :],
                            kt_sb[p0:p0 + 64, pr, kt * P:(kt + 1) * P],
                            qt_sb[p0:p0 + 64, pr, q0:q0 + 512],
                            start=True, stop=True)
                    p_t = p_pool.tile([P, 2, 512], F32R)
                    nc.scalar.activation(p_t[:], ss[:], EXP)
                    for hh in range(2):
                        h = 2 * pr + hh
                        nc.tensor.matmul(avs[hh][0:DH + 1, :], v_sb[:, kt, h, :],
                                         p_t[:, hh, :], start=(kt == 0),
                                         stop=(kt == KTN - 1))
                for hh in range(2):
                    p0 = hh * 64
                    rc = r_pool.tile([1, 512], F32)
                    nc.vector.reciprocal(rc[:], avs[hh][DH:DH + 1, :])
                    rcb = rb_pool.tile([DH, 512], F32)
                    nc.gpsimd.partition_broadcast(rcb[:], rc[:])
                    nc.vector.tensor_mul(ot_t[p0:p0 + 64, pr, :],
                                         avs[hh][0:DH, :], rcb[:])
            for tt in range(4):
                y_t = y_pool.tile([P, DIM], F32)
                for eb in range(2):
                    yp = y_ps.tile([P, 512], F32)
                    for ct in range(CT4):
                        nc.tensor.matmul(yp[:], ot_t[:, ct, tt * P:(tt + 1) * P],
                                         wo_sb[:, ct, eb * 512:(eb + 1) * 512],
                                         start=(ct == 0), stop=(ct == CT4 - 1))
                    nc.vector.tensor_copy(y_t[:, eb * 512:(eb + 1) * 512], yp[:])
                nc.sync.dma_start(y.ap()[q0 + tt * P:q0 + (tt + 1) * P, :], y_t[:])


_CACHE = {}


def _get_nc(reps: int = 1, loop: bool = False):
    key = (reps, loop)
    if key not in _CACHE:
        _CACHE[key] = _build(reps, loop)
    return _CACHE[key]


def make_in_maps(q, k, v, wq, bq, wk, bk, wv, bv, wo, bo):
    """Host-side sharding + layout prep. Returns (in_maps, const_vec)."""
    q = np.asarray(q, np.float32); k = np.asarray(k, np.float32)
    v = np.asarray(v, np.float32)
    wq = np.asarray(wq, np.float32); wk = np.asarray(wk, np.float32)
    wv = np.asarray(wv, np.float32); wo = np.asarray(wo, np.float32)
    bq = np.asarray(bq, np.float32); bv = np.asarray(bv, np.float32)
    bo = np.asarray(bo, np.float32)

    xq_b = [_round_f32r(q[b].T) for b in range(B)]
    xk_b = [_round_f32r(k[b].T) for b in range(B)]
    xv_b = [_round_f32r(v[b].T) for b in range(B)]
    ones_arr = np.ones((1, KTN * HG), np.float32)
    wqt_g, wkt_g, wvt_g, wot_g, bq_g = [], [], [], [], []
    for g in range(2):
        gs = slice(g * CD, (g + 1) * CD)
        wqt_g.append(_round_f32r((wq[gs] * SCALE).T))
        wkt_g.append(_round_f32r(wk[gs].T))
        wvt_g.append(_round_f32r(wv[gs].T))
        wot_g.append(_round_f32r(wo[:, gs].T))
        bq_g.append(np.ascontiguousarray(bq[gs] * SCALE))

    in_maps = []
    for c in range(8):
        b, g = c // 2, c % 2
        in_maps.append({
            "xq": xq_b[b], "xk": xk_b[b], "xv": xv_b[b],
            "wqt": wqt_g[g], "wkt": wkt_g[g], "wvt": wvt_g[g],
            "wot": wot_g[g], "bqs": bq_g[g], "ones": ones_arr,
        })
    const_vec = (bv.astype(np.float64) @ wo.astype(np.float64).T
                 + bo.astype(np.float64)).astype(np.float32)
    return in_maps, const_vec


def kernel(q, k, v, wq, bq, wk, bk, wv, bv, wo, bo):
    nc = _get_nc(1)
    in_maps, const_vec = make_in_maps(q, k, v, wq, bq, wk, bk, wv, bv, wo, bo)
    res = run_bass_kernel_spmd(nc, in_maps, core_ids=list(range(8)))
    out = np.empty((B, N, DIM), np.float32)
    for b in range(B):
        out[b] = res.results[2 * b]["y"] + res.results[2 * b + 1]["y"] + const_vec
    return out
